# revision 1
# baseline (speedup 1.0000x reference)
"""Causal self-attention (GQA + RoPE) Trainium2 kernel.

Sharding: 8 cores = 4 batches x 2 query-shards. Core (b, j) handles batch b
and query rows {j, j+2, j+4, ...} (stride-2 interleave -> perfectly balanced
causal work). K/V are computed over the full 2048-row prefix on both cores of
a batch pair (duplicated; avoids collectives). All per-core differences are
data (x shard, rope tables, causal masks), so one SPMD program serves all 8.

Per-core pipeline:
  1. DMA-transpose x -> xT tiles (d on partitions), bf16.
  2. K/V projection (PE, bf16), RoPE on K (DVE, natural layout), V -> SBUF
     with a ones column appended ([V|1]).
  3. Q projection from a separately-sharded xq input, RoPE, then
     DMA-transpose roped Q/K heads into qT/kT (d on partitions).
  4. Attention per head-slot: S^T = kT.T @ qT blocks (PSUM), exp on ACT
     (scale=1/8 folded, no max subtraction needed -- scores are bounded),
     0/1 mask multiply on diagonal blocks, PV matmul with lhsT=[V|1] which
     accumulates O^T rows 0..63 and the softmax denominator in row 64.
  5. Normalize O^T by the broadcast reciprocal denominator -> oT (bf16).
  6. Output projection (PE) with per-slot-packed wo, -> out [1024, 960] f32.

Head-slot permutation: q-head h -> slot s so that each slot's partition
offset (64*(s%2)) matches its kv head's kT offset (64*(g%2), g=h//3); g4 is
duplicated at both offsets to cover slot 13. Slot 15 is a zero-padded dummy.
"""

import sys

if "/opt/trn_rl_repo" not in sys.path:
    sys.path.insert(0, "/opt/trn_rl_repo")

import numpy as np
import ml_dtypes

import concourse.bass as bass
import concourse.tile as tile
from concourse import bacc, mybir
from concourse.bass_utils import run_bass_kernel_spmd

BF16 = ml_dtypes.bfloat16

B, T, DIM = 4, 2048, 960
N_HEADS, N_KV_HEADS, HEAD_DIM = 15, 5, 64
DPAD = 1024          # padded model dim (zeros in cols/rows 960:1024)
NSLOT = 16           # q-head slots (15 real + 1 dummy)
TQ = 1024            # local query rows per core
NQT = TQ // 128      # 8 q-tiles
NKT = T // 128       # 16 k-blocks
SCALE = 1.0 / 8.0    # 1/sqrt(HEAD_DIM)

# q-head for each slot; chosen so 64*(s%2) == 64*((h//3)%2) except s=13 (g4 dup)
SLOT_HEAD = [0, 3, 1, 4, 2, 5, 6, 9, 7, 10, 8, 11, 12, 13, 14, None]

_CACHE = {}


def _build_program(phases=("kv", "q", "att", "out"), rep=1, fake_t=False):
    if isinstance(phases, dict):
        reps = phases
    else:
        reps = {p: rep for p in phases}
    nc = bacc.Bacc("TRN2", target_bir_lowering=False, debug=False,
                   enable_asserts=False)
    f32 = mybir.dt.float32
    bf = mybir.dt.bfloat16

    x_d = nc.dram_tensor("x", [T, DPAD], bf, kind="ExternalInput").ap()
    xq_d = nc.dram_tensor("xq", [TQ, DPAD], bf, kind="ExternalInput").ap()
    wq_d = nc.dram_tensor("wq", [DPAD, NSLOT * HEAD_DIM], bf, kind="ExternalInput").ap()
    wkv_d = nc.dram_tensor("wkv", [DPAD, 640], bf, kind="ExternalInput").ap()
    wo_d = nc.dram_tensor("wo", [DPAD, DIM], bf, kind="ExternalInput").ap()
    cosq_d = nc.dram_tensor("cosq", [TQ, 32], f32, kind="ExternalInput").ap()
    sinq_d = nc.dram_tensor("sinq", [TQ, 32], f32, kind="ExternalInput").ap()
    cosk_d = nc.dram_tensor("cosk", [T, 32], f32, kind="ExternalInput").ap()
    sink_d = nc.dram_tensor("sink", [T, 32], f32, kind="ExternalInput").ap()
    mask_d = nc.dram_tensor("maskT", [2, 128, 128], bf, kind="ExternalInput").ap()
    out_d = nc.dram_tensor("out", [TQ, DIM], f32, kind="ExternalOutput").ap()
    # scratch for the per-head denominator broadcast (SBUF->DRAM->SBUF)
    lscr_d = nc.dram_tensor("lscratch", [NSLOT - 1, TQ], f32, kind="Internal").ap()

    def bc(ap, n, axis):
        """Insert a stride-0 broadcast dim of size n at free-dim position axis."""
        a = list(ap.ap)
        a.insert(axis, [0, n])
        return bass.AP(tensor=ap.tensor, offset=ap.offset, ap=a)

    with tile.TileContext(nc) as tc:
        with (
            tc.tile_pool(name="consts", bufs=1) as consts,
            tc.tile_pool(name="xt", bufs=3) as xtp,
            tc.tile_pool(name="rope", bufs=3) as ropep,
            tc.tile_pool(name="tmp", bufs=4) as tmpp,
            tc.tile_pool(name="pt", bufs=3) as ptp,
            tc.tile_pool(name="lnorm", bufs=2) as lnp,
            tc.tile_pool(name="ost", bufs=3) as ostp,
            tc.tile_pool(name="ps", bufs=4, space="PSUM") as psp,
        ):
            # ---- persistent SBUF tensors ----
            wq_sb = consts.tile([128, 8, NSLOT * HEAD_DIM], bf)
            wkv_sb = consts.tile([128, 8, 640], bf)
            wo_sb = consts.tile([128, 8, DIM], bf)
            cosq_sb = consts.tile([128, NQT, 32], f32)
            sinq_sb = consts.tile([128, NQT, 32], f32)
            cosk_sb = consts.tile([128, NKT, 32], f32)
            sink_sb = consts.tile([128, NKT, 32], f32)
            mask_sb = consts.tile([128, 2, 128], bf)
            qT_sb = consts.tile([128, 8, TQ], bf)
            kT_sb = consts.tile([128, 3, T], bf)
            v_sb = consts.tile([128, NKT, N_KV_HEADS, HEAD_DIM + 1], bf)
            oT_sb = consts.tile([128, 8, TQ], bf)

            nc.sync.dma_start(out=wq_sb, in_=wq_d.rearrange("(a b) c -> b a c", a=8))
            nc.sync.dma_start(out=wkv_sb, in_=wkv_d.rearrange("(a b) c -> b a c", a=8))
            nc.sync.dma_start(out=wo_sb, in_=wo_d.rearrange("(a b) c -> b a c", a=8))
            nc.sync.dma_start(out=cosq_sb, in_=cosq_d.rearrange("(a b) c -> b a c", a=NQT))
            nc.sync.dma_start(out=sinq_sb, in_=sinq_d.rearrange("(a b) c -> b a c", a=NQT))
            nc.sync.dma_start(out=cosk_sb, in_=cosk_d.rearrange("(a b) c -> b a c", a=NKT))
            nc.sync.dma_start(out=sink_sb, in_=sink_d.rearrange("(a b) c -> b a c", a=NKT))
            nc.sync.dma_start(out=mask_sb, in_=mask_d.rearrange("a b c -> b a c"))
            nc.vector.memset(v_sb[:, :, :, HEAD_DIM:HEAD_DIM + 1], 1.0)
            nc.vector.memset(oT_sb[64:128, 7, :], 0.0)  # dummy slot 15 region

            # ---- K/V projection + K rope + transposes, per k row-tile ----
            for ti in [t_ for _ in range(reps.get("kv", 0)) for t_ in range(NKT)]:
                xT = xtp.tile([128, 8, 128], bf, tag="xT")
                for db in range(8):
                    if fake_t:
                        nc.sync.dma_start(
                            out=xT[:, db, :],
                            in_=x_d[ti * 128:(ti + 1) * 128, db * 128:(db + 1) * 128])
                    else:
                        nc.sync.dma_start_transpose(
                            out=xT[:, db, :],
                            in_=x_d[ti * 128:(ti + 1) * 128, db * 128:(db + 1) * 128])
                kv_ps = psp.tile([128, 640], f32, tag="big")
                for kt in range(8):
                    nc.tensor.matmul(kv_ps[:, 0:512], xT[:, kt, :],
                                     wkv_sb[:, kt, 0:512],
                                     start=(kt == 0), stop=(kt == 7))
                    nc.tensor.matmul(kv_ps[:, 512:640], xT[:, kt, :],
                                     wkv_sb[:, kt, 512:640],
                                     start=(kt == 0), stop=(kt == 7))
                # rope K (natural layout): slots are kv heads 0..4 + dup of 4
                k_rope = ropep.tile([128, 6, HEAD_DIM], bf, tag="krope")
                ue = bass.AP(tensor=kv_ps.tensor, offset=kv_ps.offset,
                             ap=[kv_ps.ap[0], [HEAD_DIM, N_KV_HEADS], [2, 32]])
                uo = bass.AP(tensor=kv_ps.tensor, offset=kv_ps.offset + 1,
                             ap=[kv_ps.ap[0], [HEAD_DIM, N_KV_HEADS], [2, 32]])
                cb = bc(cosk_sb[:, ti, :], N_KV_HEADS, 1)
                sb_ = bc(sink_sb[:, ti, :], N_KV_HEADS, 1)
                t1 = tmpp.tile([128, N_KV_HEADS, 32], f32, tag="t1")
                t2 = tmpp.tile([128, N_KV_HEADS, 32], f32, tag="t2")
                kre = bass.AP(tensor=k_rope.tensor, offset=k_rope.offset,
                              ap=[k_rope.ap[0], [HEAD_DIM, N_KV_HEADS], [2, 32]])
                kro = bass.AP(tensor=k_rope.tensor, offset=k_rope.offset + 1,
                              ap=[k_rope.ap[0], [HEAD_DIM, N_KV_HEADS], [2, 32]])
                nc.vector.tensor_mul(t1, ue, cb)
                nc.vector.tensor_mul(t2, uo, sb_)
                nc.vector.tensor_sub(kre, t1, t2)
                nc.vector.tensor_mul(t1, ue, sb_)
                nc.vector.tensor_mul(t2, uo, cb)
                nc.vector.tensor_add(kro, t1, t2)
                nc.vector.tensor_copy(k_rope[:, 5, :], k_rope[:, 4, :])  # g4 dup
                # V -> SBUF with ones column
                nc.vector.tensor_copy(
                    v_sb[:, ti, :, 0:HEAD_DIM],
                    kv_ps[:, 320:640].rearrange("p (g d) -> p g d", g=N_KV_HEADS))
                # kT via DMA transpose, head pairs
                for tau in range(3):
                    if fake_t:
                        nc.sync.dma_start(
                            out=kT_sb[:, tau, ti * 128:(ti + 1) * 128],
                            in_=k_rope[:, 2 * tau:2 * tau + 2, :])
                    else:
                        nc.sync.dma_start_transpose(
                            out=kT_sb[:, tau, ti * 128:(ti + 1) * 128],
                            in_=k_rope[:, 2 * tau:2 * tau + 2, :])

            # ---- Q projection + rope + transposes, per q-tile ----
            for qt in [t_ for _ in range(reps.get("q", 0)) for t_ in range(NQT)]:
                xTq = xtp.tile([128, 8, 128], bf, tag="xT")
                for db in range(8):
                    if fake_t:
                        nc.sync.dma_start(
                            out=xTq[:, db, :],
                            in_=xq_d[qt * 128:(qt + 1) * 128, db * 128:(db + 1) * 128])
                    else:
                        nc.sync.dma_start_transpose(
                            out=xTq[:, db, :],
                            in_=xq_d[qt * 128:(qt + 1) * 128, db * 128:(db + 1) * 128])
                q_ps = psp.tile([128, NSLOT * HEAD_DIM], f32, tag="big")
                for kt in range(8):
                    nc.tensor.matmul(q_ps[:, 0:512], xTq[:, kt, :],
                                     wq_sb[:, kt, 0:512],
                                     start=(kt == 0), stop=(kt == 7))
                    nc.tensor.matmul(q_ps[:, 512:1024], xTq[:, kt, :],
                                     wq_sb[:, kt, 512:1024],
                                     start=(kt == 0), stop=(kt == 7))
                q_rope = ropep.tile([128, NSLOT, HEAD_DIM], bf, tag="qrope")
                ue = bass.AP(tensor=q_ps.tensor, offset=q_ps.offset,
                             ap=[q_ps.ap[0], [HEAD_DIM, NSLOT], [2, 32]])
                uo = bass.AP(tensor=q_ps.tensor, offset=q_ps.offset + 1,
                             ap=[q_ps.ap[0], [HEAD_DIM, NSLOT], [2, 32]])
                cb = bc(cosq_sb[:, qt, :], NSLOT, 1)
                sb_ = bc(sinq_sb[:, qt, :], NSLOT, 1)
                t1 = tmpp.tile([128, NSLOT, 32], f32, tag="t1")
                t2 = tmpp.tile([128, NSLOT, 32], f32, tag="t2")
                qre = bass.AP(tensor=q_rope.tensor, offset=q_rope.offset,
                              ap=[q_rope.ap[0], [HEAD_DIM, NSLOT], [2, 32]])
                qro = bass.AP(tensor=q_rope.tensor, offset=q_rope.offset + 1,
                              ap=[q_rope.ap[0], [HEAD_DIM, NSLOT], [2, 32]])
                nc.vector.tensor_mul(t1, ue, cb)
                nc.vector.tensor_mul(t2, uo, sb_)
                nc.vector.tensor_sub(qre, t1, t2)
                nc.vector.tensor_mul(t1, ue, sb_)
                nc.vector.tensor_mul(t2, uo, cb)
                nc.vector.tensor_add(qro, t1, t2)
                for tau in range(8):
                    if fake_t:
                        nc.sync.dma_start(
                            out=qT_sb[:, tau, qt * 128:(qt + 1) * 128],
                            in_=q_rope[:, 2 * tau:2 * tau + 2, :])
                    else:
                        nc.sync.dma_start_transpose(
                            out=qT_sb[:, tau, qt * 128:(qt + 1) * 128],
                            in_=q_rope[:, 2 * tau:2 * tau + 2, :])

            # ---- attention per head-slot ----
            for s in [s_ for _ in range(reps.get("att", 0)) for s_ in range(NSLOT - 1)]:
                h = SLOT_HEAD[s]
                g = h // 3
                qoff = 64 * (s % 2)
                if 64 * (g % 2) == qoff:
                    ktau, koff = g // 2, 64 * (g % 2)
                else:
                    assert g == 4
                    ktau, koff = 2, 64  # duplicated g4
                oT_ps = psp.tile([128, TQ], f32, tag="big")
                for kb in range(NKT):
                    q0 = 128 * (kb // 2)
                    sT = psp.tile([128, TQ], f32, tag="big")
                    chunks = ([(q0, 512), (512, 1024)] if q0 < 512
                              else [(q0, 1024)])
                    for (c0, c1) in chunks:
                        nc.tensor.matmul(
                            sT[:, c0:c1],
                            kT_sb[koff:koff + 64, ktau, kb * 128:(kb + 1) * 128],
                            qT_sb[qoff:qoff + 64, s // 2, c0:c1],
                            start=True, stop=True)
                    pT = ptp.tile([128, TQ], bf, tag="pT")
                    nc.scalar.activation(pT[:, q0:TQ], sT[:, q0:TQ],
                                         mybir.ActivationFunctionType.Exp,
                                         bias=0.0, scale=SCALE)
                    # causal mask on the diagonal q-tile of this k-block
                    nc.vector.tensor_mul(pT[:, q0:q0 + 128], pT[:, q0:q0 + 128],
                                         mask_sb[:, kb % 2, :])
                    for (c0, c1) in chunks:
                        nc.tensor.matmul(
                            oT_ps[0:65, c0:c1],
                            v_sb[:, kb, g, :],
                            pT[:, c0:c1],
                            start=(kb == 0), stop=(kb == NKT - 1))
                # normalize: recip of row 64 (denominators), broadcast, multiply
                linv = lnp.tile([1, TQ], f32, tag="linv")
                nc.vector.reciprocal(linv, oT_ps[64:65, :])
                lbc = lnp.tile([64, TQ], f32, tag="lbc")
                nc.sync.dma_start(out=lscr_d[s:s + 1, :], in_=linv[0:1, :])
                nc.sync.dma_start(
                    out=lbc,
                    in_=bass.AP(tensor=lscr_d.tensor, offset=lscr_d.offset + s * TQ,
                                ap=[[0, 64], [1, TQ]]))
                nc.vector.tensor_mul(oT_sb[qoff:qoff + 64, s // 2, :],
                                     oT_ps[0:64, :], lbc)

            # ---- output projection ----
            for qt in [t_ for _ in range(reps.get("out", 0)) for t_ in range(NQT)]:
                o_ps = psp.tile([128, DIM], f32, tag="big")
                for kt in range(8):
                    nc.tensor.matmul(o_ps[:, 0:512], oT_sb[:, kt, qt * 128:(qt + 1) * 128],
                                     wo_sb[:, kt, 0:512],
                                     start=(kt == 0), stop=(kt == 7))
                    nc.tensor.matmul(o_ps[:, 512:960], oT_sb[:, kt, qt * 128:(qt + 1) * 128],
                                     wo_sb[:, kt, 512:960],
                                     start=(kt == 0), stop=(kt == 7))
                ost = ostp.tile([128, DIM], f32, tag="ost")
                nc.scalar.copy(ost, o_ps)
                nc.sync.dma_start(out=out_d[qt * 128:(qt + 1) * 128, :], in_=ost)
            if not reps.get("out", 0):
                ost = ostp.tile([128, DIM], f32, tag="ost")
                nc.vector.memset(ost, 0.0)
                nc.sync.dma_start(out=out_d[0:128, :], in_=ost)

    nc.finalize()
    return nc


def _host_prep(x, freqs_cos, freqs_sin, wq, wk, wv, wo):
    """Build the shared + per-core input arrays (all numpy, host-side)."""
    xp = np.zeros((B, T, DPAD), dtype=BF16)
    xp[:, :, :DIM] = x.astype(BF16)

    wqp = np.zeros((DPAD, NSLOT * HEAD_DIM), dtype=BF16)
    for s, h in enumerate(SLOT_HEAD):
        if h is None:
            continue
        wqp[:DIM, s * 64:(s + 1) * 64] = wq[:, h * 64:(h + 1) * 64].astype(BF16)

    wkvp = np.zeros((DPAD, 640), dtype=BF16)
    wkvp[:DIM, 0:320] = wk.astype(BF16)
    wkvp[:DIM, 320:640] = wv.astype(BF16)

    wop = np.zeros((DPAD, DIM), dtype=BF16)
    for s, h in enumerate(SLOT_HEAD):
        if h is None:
            continue
        r = 128 * (s // 2) + 64 * (s % 2)
        wop[r:r + 64, :] = wo[h * 64:(h + 1) * 64, :].astype(BF16)

    cosk = np.ascontiguousarray(freqs_cos, dtype=np.float32)
    sink = np.ascontiguousarray(freqs_sin, dtype=np.float32)

    shared = dict(wq=wqp, wkv=wkvp, wo=wop, cosk=cosk, sink=sink)

    in_maps = []
    for c in range(8):
        b, j = c // 2, c % 2
        m = dict(shared)
        m["x"] = np.ascontiguousarray(xp[b])
        m["xq"] = np.ascontiguousarray(xp[b, j::2])
        m["cosq"] = np.ascontiguousarray(cosk[j::2])
        m["sinq"] = np.ascontiguousarray(sink[j::2])
        kk = np.arange(128)[None, :, None]          # k index within block
        p = np.arange(128)[None, None, :]           # q row within tile
        mhalf = np.arange(2)[:, None, None] * 128
        mask = ((mhalf + kk) <= (2 * p + j)).astype(BF16)
        m["maskT"] = np.ascontiguousarray(mask)
        in_maps.append(m)
    return in_maps


def kernel(x, freqs_cos, freqs_sin, wq, wk, wv, wo):
    if "nc" not in _CACHE:
        _CACHE["nc"] = _build_program()
    nc = _CACHE["nc"]
    in_maps = _host_prep(np.asarray(x), np.asarray(freqs_cos),
                         np.asarray(freqs_sin), np.asarray(wq),
                         np.asarray(wk), np.asarray(wv), np.asarray(wo))
    res = run_bass_kernel_spmd(nc, in_maps, core_ids=list(range(8)))
    out = np.empty((B, T, DIM), dtype=np.float32)
    for c in range(8):
        b, j = c // 2, c % 2
        out[b, j::2, :] = res.results[c]["out"]
    return out



# revision 8
# speedup vs baseline: 1.4671x; 1.4671x over previous
"""Causal self-attention (GQA + RoPE) Trainium2 kernel, v2.

Sharding: 8 cores = 4 batches x 2 query-parities. Core (b, j) handles batch b
and original query rows {j, j+2, ...} (stride-2 interleave -> balanced causal
work). K/V cover the full 2048-key prefix on both cores of a batch pair.

Dispatch-overhead design (dominates the pipelined per-iter measurement):
  * ONE ExternalInput per core -- a flat bf16 blob holding x^T, all weights,
    rope tables (f32 regions read via AP.bitcast) and causal masks. Each extra
    input argument costs ~1.3 ms/iter through the axon PJRT tunnel, and input
    bytes ~0.56 ms/MB/core, so everything is packed and minimized (~9.5 MB).
  * No device-side transposes at all: the host ships x already transposed
    (d-major), Q/K are produced directly in transposed layout by using the
    weights as the stationary matmul operand, and RoPE is applied in that
    layout via an even/odd component split baked into the weight columns.

Key-order trick: each core's x^T columns are permuted [own-parity t's first,
other-parity second]. The q-shard is then a contiguous slice (cols 0:1024), Q
rope tables are a prefix of the K tables, and causality becomes two triangular
block sets with per-core 0/1 diagonal masks (all per-core differences stay in
DATA; one SPMD program serves all 8 cores).

Per-core pipeline:
  1. Big DMAs: xT/wq/wkv/wo/tables/mask blob regions -> SBUF.
  2. V = x@wv (natural, PE) -> v_sb with a ones column appended ([V|1]).
  3. K^T = wk'^T@x^T, Q^T = wq'^T@x^T (PE, transposed out), RoPE on DVE in
     transposed layout: t1=u*[c;s;c;s], t2=u*[s;c;s;c], out_e=t1_hi-t1_lo,
     out_o=t2_hi+t2_lo (component pairs live 32 partitions apart).
  4. Attention per head-slot: S^T = kT.T @ qT blocks (PSUM), exp on ACT
     (scale=1/8 folded; scores bounded, no max-sub), 0/1 mask on diagonal
     blocks, PV matmul with lhsT=[V|1] accumulating O^T + denominator row.
  5. Normalize O^T by broadcast reciprocal denominator (DRAM-scratch bcast).
  6. Output projection (PE) with slot-packed wo -> out [1024, 960] f32.
"""

import sys

if "/opt/trn_rl_repo" not in sys.path:
    sys.path.insert(0, "/opt/trn_rl_repo")

import numpy as np
import ml_dtypes

import concourse.bass as bass
import concourse.tile as tile
from concourse import bacc, mybir
from concourse.bass_utils import run_bass_kernel_spmd

BF16 = ml_dtypes.bfloat16

B, T, DIM = 4, 2048, 960
N_HEADS, N_KV_HEADS, HEAD_DIM = 15, 5, 64
TQ = 1024            # local query rows per core
NQT = TQ // 128      # 8 q-tiles
SCALE = 1.0 / 8.0    # 1/sqrt(HEAD_DIM)

# q-head per slot; chosen so each slot's partition offset 64*(s%2) matches its
# kv head's kT offset 64*(g%2) (g = h//3), with g4 duplicated to cover s=13.
SLOT_HEAD = [0, 3, 1, 4, 2, 5, 6, 9, 7, 10, 8, 11, 12, 13, 14]
KS_G = [0, 1, 2, 3, 4, 4]                     # kT slot -> kv head (g4 dup'd)
PERM = [2 * i for i in range(32)] + [2 * i + 1 for i in range(32)]

# blob regions (bf16 element offsets)
LX, LWQ, LWKV = 960 * 2048, 960 * 1024, 960 * 704
LWO, LTAB, LMASK = 960 * 960, 96 * 4096, 256 * 128
OFF_X = 0
OFF_WQ = OFF_X + LX
OFF_WKV = OFF_WQ + LWQ
OFF_WO = OFF_WKV + LWKV
OFF_TAB = OFF_WO + LWO
OFF_MASK = OFF_TAB + LTAB
LBLOB = OFF_MASK + LMASK

_CACHE = {}


def _build_program(phases=("proj", "att", "out"), rep=1):
    if isinstance(phases, dict):
        reps = phases
    else:
        reps = {p: rep for p in phases}
    nc = bacc.Bacc("TRN2", target_bir_lowering=False, debug=False,
                   enable_asserts=False)
    f32 = mybir.dt.float32
    bf = mybir.dt.bfloat16

    blob_d = nc.dram_tensor("blob", [LBLOB], bf, kind="ExternalInput").ap()
    out_d = nc.dram_tensor("out", [TQ, DIM], f32, kind="ExternalOutput").ap()
    # scratch for the per-head denominator broadcast (SBUF->DRAM->SBUF)
    lscr_d = nc.dram_tensor("lscratch", [N_HEADS, TQ], f32, kind="Internal").ap()

    def dap(off, dims, dtype=None):
        a = bass.AP(tensor=blob_d.tensor, offset=blob_d.offset + off,
                    ap=[list(d) for d in dims])
        return a.bitcast(dtype) if dtype is not None else a

    with tile.TileContext(nc) as tc:
        with (
            tc.tile_pool(name="consts", bufs=1) as consts,
            tc.tile_pool(name="rt", bufs=3) as rtp,
            tc.tile_pool(name="pt", bufs=3) as ptp,
            tc.tile_pool(name="ln", bufs=2) as lnp,
            tc.tile_pool(name="ost", bufs=2) as ostp,
            tc.tile_pool(name="ps", bufs=4, space="PSUM") as psp,
        ):
            # ---- persistent SBUF tensors ----
            xT_sb = consts.tile([128, 8, T], bf)
            wq_sb = consts.tile([128, 8, 1024], bf)
            wkv_sb = consts.tile([128, 8, 704], bf)
            wo_sb = consts.tile([128, 8, DIM], bf)
            ct_sb = consts.tile([128, T], f32)    # [c;c;c;c] pattern
            st_sb = consts.tile([128, T], f32)    # [-s;+s;-s;+s] pattern
            mask_sb = consts.tile([128, 2, 128], bf)
            kT_sb = consts.tile([128, 3, T], bf)
            qT_sb = consts.tile([128, 8, TQ], bf)
            v_sb = consts.tile([128, 16, N_KV_HEADS, HEAD_DIM + 1], bf)
            oT_sb = consts.tile([128, 8, TQ], bf)

            # ---- blob -> SBUF loads (d rows 0:960; block 7 parts 64:128 pad) ----
            nc.sync.dma_start(out=xT_sb[:, 0:7, :],
                              in_=dap(OFF_X, [[T, 128], [128 * T, 7], [1, T]]))
            nc.sync.dma_start(out=xT_sb[0:64, 7, :],
                              in_=dap(OFF_X + 896 * T, [[T, 64], [1, T]]))
            nc.sync.dma_start(out=wkv_sb[:, 0:7, :],
                              in_=dap(OFF_WKV, [[704, 128], [704 * 128, 7], [1, 704]]))
            nc.sync.dma_start(out=wkv_sb[0:64, 7, :],
                              in_=dap(OFF_WKV + 896 * 704, [[704, 64], [1, 704]]))
            nc.sync.dma_start(out=wq_sb[:, 0:7, :],
                              in_=dap(OFF_WQ, [[1024, 128], [1024 * 128, 7], [1, 1024]]))
            nc.sync.dma_start(out=wq_sb[0:64, 7, :],
                              in_=dap(OFF_WQ + 896 * 1024, [[1024, 64], [1, 1024]]))
            for pb in (0, 32, 64, 96):
                nc.sync.dma_start(out=ct_sb[pb:pb + 32, :],
                                  in_=dap(OFF_TAB, [[4096, 32], [1, 4096]], f32))
            for pb, row in ((0, 64), (32, 32), (64, 64), (96, 32)):
                nc.sync.dma_start(out=st_sb[pb:pb + 32, :],
                                  in_=dap(OFF_TAB + row * 4096,
                                          [[4096, 32], [1, 4096]], f32))
            nc.sync.dma_start(out=mask_sb,
                              in_=dap(OFF_MASK, [[128, 128], [128 * 128, 2], [1, 128]]))
            nc.sync.dma_start(out=wo_sb[:, 0:7, :],
                              in_=dap(OFF_WO, [[DIM, 128], [DIM * 128, 7], [1, DIM]]))
            nc.sync.dma_start(out=wo_sb[0:64, 7, :],
                              in_=dap(OFF_WO + 896 * DIM, [[DIM, 64], [1, DIM]]))
            nc.vector.memset(xT_sb[64:128, 7, :], 0.0)
            nc.vector.memset(wq_sb[64:128, 7, :], 0.0)
            nc.vector.memset(wkv_sb[64:128, 7, :], 0.0)
            nc.vector.memset(wo_sb[64:128, 7, :], 0.0)
            nc.vector.memset(v_sb[:, :, :, HEAD_DIM:HEAD_DIM + 1], 1.0)
            nc.vector.memset(oT_sb[64:128, 7, :], 0.0)  # dummy slot 15

            def rope(dst, src, c0, c1):
                """RoPE in transposed layout: dst[128, c1-c0] (SBUF bf16) from
                src (PSUM f32 [128, TQ] tile): out = u*C + swap32(u)*S± where
                component pairs live 32 partitions apart (SB*SB DVE ops must
                share a base partition, so the swap is materialized)."""
                n = c1 - c0
                usw = rtp.tile([128, 512], f32, tag="usw")
                m1 = rtp.tile([128, 512], f32, tag="m1")
                m2 = rtp.tile([128, 512], f32, tag="m2")
                for hb in (0, 64):
                    nc.vector.tensor_copy(usw[hb:hb + 32, 0:n],
                                          src[hb + 32:hb + 64, 0:n])
                    nc.vector.tensor_copy(usw[hb + 32:hb + 64, 0:n],
                                          src[hb:hb + 32, 0:n])
                nc.vector.tensor_mul(m1[:, 0:n], src[:, 0:n], ct_sb[:, c0:c1])
                nc.vector.tensor_mul(m2[:, 0:n], usw[:, 0:n], st_sb[:, c0:c1])
                nc.vector.tensor_add(dst, m1[:, 0:n], m2[:, 0:n])

            # ---- projections ----
            for _ in range(reps.get("proj", 0)):
                # V (natural layout) per key tile
                for tt in range(16):
                    v_ps = psp.tile([128, TQ], f32, tag="big")
                    for a in range(8):
                        nc.tensor.matmul(v_ps[:, 0:320],
                                         xT_sb[:, a, tt * 128:(tt + 1) * 128],
                                         wkv_sb[:, a, 384:704],
                                         start=(a == 0), stop=(a == 7))
                    nc.vector.tensor_copy(
                        v_sb[:, tt, :, 0:HEAD_DIM],
                        v_ps[:, 0:320].rearrange("p (g d) -> p g d",
                                                 g=N_KV_HEADS))
                # K^T + rope
                for tau in range(3):
                    for ch in range(4):
                        c0 = ch * 512
                        kT_ps = psp.tile([128, TQ], f32, tag="big")
                        for a in range(8):
                            nc.tensor.matmul(kT_ps[:, 0:512],
                                             wkv_sb[:, a, tau * 128:(tau + 1) * 128],
                                             xT_sb[:, a, c0:c0 + 512],
                                             start=(a == 0), stop=(a == 7))
                        rope(kT_sb[:, tau, c0:c0 + 512], kT_ps, c0, c0 + 512)
                # Q^T + rope (q cols = own half = table cols 0:1024)
                for beta in range(8):
                    for ch in range(2):
                        c0 = ch * 512
                        qT_ps = psp.tile([128, TQ], f32, tag="big")
                        for a in range(8):
                            nc.tensor.matmul(qT_ps[:, 0:512],
                                             wq_sb[:, a, beta * 128:(beta + 1) * 128],
                                             xT_sb[:, a, c0:c0 + 512],
                                             start=(a == 0), stop=(a == 7))
                        rope(qT_sb[:, beta, c0:c0 + 512], qT_ps, c0, c0 + 512)

            # ---- attention per head-slot ----
            for s in [s_ for _ in range(reps.get("att", 0))
                      for s_ in range(N_HEADS)]:
                h = SLOT_HEAD[s]
                g = h // 3
                beta, qoff = s // 2, 64 * (s % 2)
                if 64 * (g % 2) == qoff:
                    ktau, koff = g // 2, qoff
                else:
                    ktau, koff = 2, 64  # duplicated g4
                oT_ps = psp.tile([128, TQ], f32, tag="big")
                for kb in range(16):
                    half, kbi = kb // 8, kb % 8
                    q0 = 128 * kbi
                    kcol = half * TQ + q0
                    sT = psp.tile([128, TQ], f32, tag="big")
                    chunks = ([(q0, 512), (512, TQ)] if q0 < 512
                              else [(q0, TQ)])
                    for (c0, c1) in chunks:
                        nc.tensor.matmul(
                            sT[:, c0:c1],
                            kT_sb[koff:koff + 64, ktau, kcol:kcol + 128],
                            qT_sb[qoff:qoff + 64, beta, c0:c1],
                            start=True, stop=True)
                    pT = ptp.tile([128, TQ], bf, tag="pT")
                    nc.scalar.activation(pT[:, q0:TQ], sT[:, q0:TQ],
                                         mybir.ActivationFunctionType.Exp,
                                         bias=0.0, scale=SCALE)
                    nc.vector.tensor_mul(pT[:, q0:q0 + 128], pT[:, q0:q0 + 128],
                                         mask_sb[:, half, :])
                    # PSUM group start/stop is per 2KB zero region (= 512 f32
                    # cols): region 0 is last written at kb=11 (diag 384:512),
                    # region 1 at kb=15.
                    for (c0, c1) in chunks:
                        nc.tensor.matmul(
                            oT_ps[0:65, c0:c1],
                            v_sb[:, kb, g, :],
                            pT[:, c0:c1],
                            start=(kb == 0),
                            stop=(kb == 11 if c1 <= 512 else kb == 15))
                # normalize by the denominator row (64), bcast via DRAM
                linv = lnp.tile([1, TQ], f32, tag="linv")
                nc.vector.reciprocal(linv, oT_ps[64:65, :])
                lbc = lnp.tile([64, TQ], f32, tag="lbc")
                nc.sync.dma_start(out=lscr_d[s:s + 1, :], in_=linv[0:1, :])
                nc.sync.dma_start(
                    out=lbc,
                    in_=bass.AP(tensor=lscr_d.tensor,
                                offset=lscr_d.offset + s * TQ,
                                ap=[[0, 64], [1, TQ]]))
                nc.vector.tensor_mul(oT_sb[qoff:qoff + 64, beta, :],
                                     oT_ps[0:64, :], lbc)

            # ---- output projection ----
            for qt in [t_ for _ in range(reps.get("out", 0))
                       for t_ in range(NQT)]:
                o_ps = psp.tile([128, TQ], f32, tag="big")
                for a in range(8):
                    nc.tensor.matmul(o_ps[:, 0:512],
                                     oT_sb[:, a, qt * 128:(qt + 1) * 128],
                                     wo_sb[:, a, 0:512],
                                     start=(a == 0), stop=(a == 7))
                    nc.tensor.matmul(o_ps[:, 512:DIM],
                                     oT_sb[:, a, qt * 128:(qt + 1) * 128],
                                     wo_sb[:, a, 512:DIM],
                                     start=(a == 0), stop=(a == 7))
                ost = ostp.tile([128, DIM], f32, tag="ost")
                nc.scalar.copy(ost, o_ps[:, 0:DIM])
                nc.sync.dma_start(out=out_d[qt * 128:(qt + 1) * 128, :], in_=ost)
            if not reps.get("out", 0):
                ost = ostp.tile([128, DIM], f32, tag="ost")
                nc.vector.memset(ost, 0.0)
                nc.sync.dma_start(out=out_d[0:128, :], in_=ost)

    nc.finalize()
    return nc


def _host_prep(x, freqs_cos, freqs_sin, wq, wk, wv, wo):
    """Build the single packed blob per core (all numpy, host-side)."""
    x = np.asarray(x, np.float32)
    cos = np.asarray(freqs_cos, np.float32)
    sin = np.asarray(freqs_sin, np.float32)
    perm = np.array(PERM)

    wqp = np.zeros((960, 1024), np.float32)
    for s, h in enumerate(SLOT_HEAD):
        wqp[:, 64 * s:64 * s + 64] = np.asarray(wq)[:, 64 * h + perm]
    wkvp = np.zeros((960, 704), np.float32)
    for ks, gk in enumerate(KS_G):
        wkvp[:, 64 * ks:64 * ks + 64] = np.asarray(wk)[:, 64 * gk + perm]
    wkvp[:, 384:704] = np.asarray(wv)
    wop = np.zeros((960, 960), np.float32)
    for s, h in enumerate(SLOT_HEAD):
        r = 128 * (s // 2) + 64 * (s % 2)
        wop[r:r + 64, :] = np.asarray(wo)[64 * h:64 * h + 64, :]
    wq_flat = wqp.astype(BF16).reshape(-1)
    wkv_flat = wkvp.astype(BF16).reshape(-1)
    wo_flat = wop.astype(BF16).reshape(-1)

    kk = np.arange(128)[:, None]
    rr = np.arange(128)[None, :]

    in_maps = []
    for c in range(8):
        b, j = c // 2, c % 2
        xp = x[b]                                     # [2048, 960]
        xperm = np.concatenate([xp[j::2], xp[1 - j::2]], axis=0)
        xT = xperm.T.astype(BF16)                     # [960, 2048] contiguous
        cosp = np.concatenate([cos[j::2], cos[1 - j::2]], 0).T  # [32, 2048]
        sinp = np.concatenate([sin[j::2], sin[1 - j::2]], 0).T
        tab = np.ascontiguousarray(
            np.concatenate([cosp, sinp, -sinp], 0))   # [96, 2048] f32
        tab_flat = tab.view(BF16).reshape(-1)         # raw bytes as bf16
        m0 = (kk <= rr).astype(BF16)                  # own-parity diagonal
        m1 = ((kk + (1 - j)) <= rr).astype(BF16)      # other-parity diagonal
        mask_flat = np.stack([m0, m1], 0).reshape(-1)

        blob = np.empty(LBLOB, BF16)
        blob[OFF_X:OFF_X + LX] = xT.reshape(-1)
        blob[OFF_WQ:OFF_WQ + LWQ] = wq_flat
        blob[OFF_WKV:OFF_WKV + LWKV] = wkv_flat
        blob[OFF_WO:OFF_WO + LWO] = wo_flat
        blob[OFF_TAB:OFF_TAB + LTAB] = tab_flat
        blob[OFF_MASK:OFF_MASK + LMASK] = mask_flat
        in_maps.append({"blob": blob})
    return in_maps


def kernel(x, freqs_cos, freqs_sin, wq, wk, wv, wo):
    if "nc" not in _CACHE:
        _CACHE["nc"] = _build_program()
    nc = _CACHE["nc"]
    in_maps = _host_prep(x, freqs_cos, freqs_sin, wq, wk, wv, wo)
    res = run_bass_kernel_spmd(nc, in_maps, core_ids=list(range(8)))
    out = np.empty((B, T, DIM), dtype=np.float32)
    for c in range(8):
        b, j = c // 2, c % 2
        out[b, j::2, :] = res.results[c]["out"]
    return out
